# revision 1
# baseline (speedup 1.0000x reference)
"""CrossAttention Trainium2 kernel, 8-core SPMD, fp32r hi/lo split precision.

Sharding: core c -> (batch b = c//2, head-group g = c%2).  Each core computes
8 of the 16 heads for one batch: q/k/v projections restricted to its
inner-dim slice [g*512:(g+1)*512], full attention for those heads, and a
partial output projection (contraction over its 512 inner dims).  Host
pre-transposes x/context, pre-splits every input into an exact
fp32 = fp32r_hi + fp32r_lo pair, and sums the two partial outputs per
batch + bias.

Precision scheme: every matmul runs on the PE in float32r (1 cycle/row vs 4
for native fp32).  A fp32 value splits EXACTLY into hi + lo fp32r parts
(12-bit mantissas); a@b is computed as ah@bh + ah@bl + al@bh (the dropped
al@bl term is ~2^-24 relative).  Products of fp32r operands accumulate
exactly in fp32 PSUM, so the result is fp32-grade (~5e-6 scale-rel, HW
validated) at 3 cycles/row = 0.75x native-fp32 cost; the K=64 sim matmul
packs its two cross terms into one K=128 pass (2 passes total).

Per-core dataflow:
  Q^T[inner,n] = Wq_s^T @ x^T   (scale folded into Wq) -> split -> DRAM qtb
                 per head: rows [q_hi(0:64); q_lo(64:128)]
  K^T[inner,m] -> split -> DRAM ktb, per head rows [k_hi; k_lo]
  V[m,inner]   -> split into vah/val [128, 8*65] tiles; col h*65+64 is 1.0
                 in vah / 0.0 in val (softmax denominator rides the matmul)
  per head h, per n-chunk j (512):
    simT = 2-pass split matmul (PSUM, partition = m)
    e = exp(simT): ef (fp32) + eh (fp32r, ACT rounds) + el = ef - eh
    oT[65,n] = 24-mm split attn@v; row 64 = denominator
    recip = 1/oT[64] -> rh + rl -> K=1 ones matmul broadcast -> pbs
    ao = oT[0:64] * pbs -> split into aoh/aol tiles
  out_part[n,1024] = 3-pass split aoT.T @ Wo_s   (host adds pair + bias)
"""
import sys

sys.path.insert(0, "/opt/trn_rl_repo")

import numpy as np

import concourse.bacc as bacc
import concourse.mybir as mybir
import concourse.tile as tile
from concourse.bass_utils import run_bass_kernel_spmd

# bass_utils imports antenv.axon_hooks when trace=True; the read-only antenv
# package in this image lacks it, so register a no-op stub if missing.
try:
    import antenv.axon_hooks  # noqa: F401
except ImportError:
    import types as _types

    _stub = _types.ModuleType("antenv.axon_hooks")
    _stub.get_axon_ntff_profile_hook = lambda: None
    _stub.set_axon_ntff_profile_hook = lambda h: None
    sys.modules["antenv.axon_hooks"] = _stub

F32 = mybir.dt.float32
F32R = mybir.dt.float32r
EXP = mybir.ActivationFunctionType.Exp

B, N, M = 4, 2048, 1024
QD, CD = 1024, 768
HEADS, DH = 16, 64
INNER = HEADS * DH
HG = 8            # heads per core
IS = HG * DH      # inner slice per core = 512
NC = 8

LAST_RESULTS = None  # stashed BassKernelResults for test.py introspection


def _r(x):
    """Round fp32 -> nearest fp32r (11 explicit mantissa bits)."""
    b = np.ascontiguousarray(x, np.float32).view(np.uint32)
    return (((b.astype(np.uint64) + 0x800) & 0xFFFFF000)
            .astype(np.uint32)).view(np.float32)


def build_nc():
    nc = bacc.Bacc("TRN2", target_bir_lowering=False, debug=False, num_devices=NC)

    def din(name, shape):
        return nc.dram_tensor(name, shape, F32R, kind="ExternalInput").ap()

    xTh, xTl = din("xTh", [QD, N]), din("xTl", [QD, N])
    cTh, cTl = din("cTh", [CD, M]), din("cTl", [CD, M])
    wqh, wql = din("wqh", [QD, IS]), din("wql", [QD, IS])
    wkh, wkl = din("wkh", [CD, IS]), din("wkl", [CD, IS])
    wvh, wvl = din("wvh", [CD, IS]), din("wvl", [CD, IS])
    woh, wol = din("woh", [IS, QD]), din("wol", [IS, QD])
    out = nc.dram_tensor("out", [N, QD], F32, kind="ExternalOutput").ap()
    qtb = nc.dram_tensor("qtb", [HG, 128, N], F32R).ap()   # [q_hi; q_lo] rows
    aob = nc.dram_tensor("aob", [2 * IS, N], F32R).ap()    # attnout hi(0:512)/lo(512:)

    KQ = QD // 128
    KC = CD // 128
    NJ = N // 512
    MT = M // 128
    IT = IS // 128

    with tile.TileContext(nc) as tc:
        with tc.tile_pool(name="va", bufs=1) as vap, \
             tc.tile_pool(name="sp", bufs=3) as spp, \
             tc.tile_pool(name="sm", bufs=3) as smp, \
             tc.tile_pool(name="os", bufs=2) as osp, \
             tc.tile_pool(name="ps", bufs=6, space="PSUM") as psp, \
             tc.tile_pool(name="po", bufs=2, space="PSUM") as pop:

            # 3-pass split accumulation into one PSUM tile
            def split_mm(ps, hil, lol, rh, rl, nk):
                idx, last = 0, 3 * nk - 1
                for (ws, xs) in ((hil, rh), (hil, rl), (lol, rh)):
                    for k in range(nk):
                        nc.tensor.matmul(ps[:], ws[k], xs[k],
                                         start=(idx == 0), stop=(idx == last))
                        idx += 1

            # ---------------- Q^T -> qtb ----------------
            with nc.named_scope("qproj"), \
                 tc.tile_pool(name="wqp", bufs=1) as wqp, \
                 tc.tile_pool(name="xs", bufs=2) as xsp:
                wqh_sb = [wqp.tile([128, IS], F32R, tag=f"wqh{k}", name=f"wqh{k}")
                          for k in range(KQ)]
                wql_sb = [wqp.tile([128, IS], F32R, tag=f"wql{k}", name=f"wql{k}")
                          for k in range(KQ)]
                for k in range(KQ):
                    nc.sync.dma_start(wqh_sb[k][:], wqh[k * 128:(k + 1) * 128, :])
                    nc.sync.dma_start(wql_sb[k][:], wql[k * 128:(k + 1) * 128, :])
                for jn in range(NJ):
                    nsl = slice(jn * 512, (jn + 1) * 512)
                    xh = [xsp.tile([128, 512], F32R, tag=f"xh{k}", name=f"xh{k}")
                          for k in range(KQ)]
                    xl = [xsp.tile([128, 512], F32R, tag=f"xl{k}", name=f"xl{k}")
                          for k in range(KQ)]
                    for k in range(KQ):
                        ksl = slice(k * 128, (k + 1) * 128)
                        nc.sync.dma_start(xh[k][:], xTh[ksl, nsl])
                        nc.sync.dma_start(xl[k][:], xTl[ksl, nsl])
                    for mi in range(IT):
                        isl = slice(mi * 128, (mi + 1) * 128)
                        ps = psp.tile([128, 512], F32, tag="mm", name="mm")
                        split_mm(ps,
                                 [t[:, isl] for t in wqh_sb],
                                 [t[:, isl] for t in wql_sb],
                                 [t[:] for t in xh], [t[:] for t in xl], KQ)
                        hi = spp.tile([128, 512], F32R, tag="hi", name="hi")
                        lo = spp.tile([128, 512], F32R, tag="lo", name="lo")
                        nc.vector.tensor_copy(hi[:], ps[:])
                        nc.vector.tensor_sub(lo[:], ps[:], hi[:])
                        for sub in range(2):
                            h = 2 * mi + sub
                            rsl = slice(sub * 64, sub * 64 + 64)
                            nc.sync.dma_start(qtb[h, 0:64, nsl], hi[rsl, :])
                            nc.sync.dma_start(qtb[h, 64:128, nsl], lo[rsl, :])

            with tc.tile_pool(name="kp", bufs=1) as kpp:
                # ------- K^T -> resident kct tiles [k_hi; k_lo]; V -> vaug -------
                kct_sb = [kpp.tile([128, M], F32R, tag=f"kct{h}", name=f"kct{h}")
                          for h in range(HG)]
                with nc.named_scope("kproj"), \
                     tc.tile_pool(name="wkvp", bufs=1) as wkvp, \
                     tc.tile_pool(name="cs", bufs=2) as csp:
                    wkh_sb = [wkvp.tile([128, IS], F32R, tag=f"wkh{k}", name=f"wkh{k}")
                              for k in range(KC)]
                    wkl_sb = [wkvp.tile([128, IS], F32R, tag=f"wkl{k}", name=f"wkl{k}")
                              for k in range(KC)]
                    for k in range(KC):
                        ksl = slice(k * 128, (k + 1) * 128)
                        nc.sync.dma_start(wkh_sb[k][:], wkh[ksl, :])
                        nc.sync.dma_start(wkl_sb[k][:], wkl[ksl, :])

                    va_h = [vap.tile([128, HG * 65], F32R, tag=f"vah{mi}",
                                     name=f"vah{mi}") for mi in range(MT)]
                    va_l = [vap.tile([128, HG * 65], F32R, tag=f"val{mi}",
                                     name=f"val{mi}") for mi in range(MT)]
                    onesf = smp.tile([128, 64], F32, tag="onesf", name="onesf")
                    nc.vector.memset(onesf[:], 1.0)
                    zerof = smp.tile([128, HG], F32, tag="zerof", name="zerof")
                    nc.vector.memset(zerof[:], 0.0)

                    for jm in range(M // 512):
                        msl = slice(jm * 512, (jm + 1) * 512)
                        ch = [csp.tile([128, 512], F32R, tag=f"ch{k}", name=f"ch{k}")
                              for k in range(KC)]
                        cl = [csp.tile([128, 512], F32R, tag=f"cl{k}", name=f"cl{k}")
                              for k in range(KC)]
                        for k in range(KC):
                            ksl = slice(k * 128, (k + 1) * 128)
                            nc.sync.dma_start(ch[k][:], cTh[ksl, msl])
                            nc.sync.dma_start(cl[k][:], cTl[ksl, msl])
                        for mi in range(IT):
                            isl = slice(mi * 128, (mi + 1) * 128)
                            ps = psp.tile([128, 512], F32, tag="mm", name="mm")
                            split_mm(ps,
                                     [t[:, isl] for t in wkh_sb],
                                     [t[:, isl] for t in wkl_sb],
                                     [t[:] for t in ch], [t[:] for t in cl], KC)
                            hi = spp.tile([128, 512], F32R, tag="hi", name="hi")
                            lo = spp.tile([128, 512], F32R, tag="lo", name="lo")
                            nc.vector.tensor_copy(hi[:], ps[:])
                            nc.vector.tensor_sub(lo[:], ps[:], hi[:])
                            for sub in range(2):
                                h = 2 * mi + sub
                                rsl = slice(sub * 64, sub * 64 + 64)
                                nc.sync.dma_start(kct_sb[h][0:64, msl], hi[rsl, :])
                                nc.sync.dma_start(kct_sb[h][64:128, msl], lo[rsl, :])

                # ---------------- V -> vaug (own scope) ----------------
                with nc.named_scope("vproj"), \
                     tc.tile_pool(name="wvp", bufs=1) as wvp, \
                     tc.tile_pool(name="cv", bufs=2) as cvp:
                    wvh_sb = [wvp.tile([128, IS], F32R, tag=f"wvh{k}", name=f"wvh{k}")
                              for k in range(KC)]
                    wvl_sb = [wvp.tile([128, IS], F32R, tag=f"wvl{k}", name=f"wvl{k}")
                              for k in range(KC)]
                    for k in range(KC):
                        ksl = slice(k * 128, (k + 1) * 128)
                        nc.sync.dma_start(wvh_sb[k][:], wvh[ksl, :])
                        nc.sync.dma_start(wvl_sb[k][:], wvl[ksl, :])
                    for jm in range(M // 512):
                        msl = slice(jm * 512, (jm + 1) * 512)
                        ch = [cvp.tile([128, 512], F32R, tag=f"vch{k}", name=f"vch{k}")
                              for k in range(KC)]
                        cl = [cvp.tile([128, 512], F32R, tag=f"vcl{k}", name=f"vcl{k}")
                              for k in range(KC)]
                        for k in range(KC):
                            ksl = slice(k * 128, (k + 1) * 128)
                            nc.sync.dma_start(ch[k][:], cTh[ksl, msl])
                            nc.sync.dma_start(cl[k][:], cTl[ksl, msl])
                        for mt in range(4):
                            mi = jm * 4 + mt
                            tsl = slice(mt * 128, (mt + 1) * 128)
                            ps = psp.tile([128, 512], F32, tag="mm", name="mm")
                            split_mm(ps,
                                     [t[:, tsl] for t in ch],
                                     [t[:, tsl] for t in cl],
                                     [t[:] for t in wvh_sb], [t[:] for t in wvl_sb],
                                     KC)
                            hcol = va_h[mi][:].rearrange("p (h c) -> p h c", c=65)
                            lcol = va_l[mi][:].rearrange("p (h c) -> p h c", c=65)
                            psv = ps[:].rearrange("p (h c) -> p h c", c=64)
                            nc.vector.tensor_copy(hcol[:, :, 0:64], psv[:])
                            nc.vector.tensor_sub(lcol[:, :, 0:64], psv[:],
                                                 hcol[:, :, 0:64])
                            ocol = hcol
                            zcol = lcol
                            nc.vector.tensor_copy(ocol[:, :, 64], onesf[:, 0:HG])
                            nc.vector.tensor_copy(zcol[:, :, 64], zerof[:])

                # ---------------- attention ----------------
                with nc.named_scope("attn"), \
                     tc.tile_pool(name="kq", bufs=3) as kqp, \
                     tc.tile_pool(name="ex", bufs=1) as exp_pool:
                    for h in range(HG):
                        kct = kct_sb[h]
                        for jn in range(NJ):
                            nsl = slice(jn * 512, (jn + 1) * 512)
                            qhl = kqp.tile([128, 512], F32R, tag="qhl", name="qhl")
                            qlh = kqp.tile([128, 512], F32R, tag="qlh", name="qlh")
                            nc.sync.dma_start(qhl[:], qtb[h, :, nsl])
                            nc.sync.dma_start(qlh[0:64, :], qtb[h, 64:128, nsl])
                            nc.sync.dma_start(qlh[64:128, :], qtb[h, 0:64, nsl])
                            es_h, es_l = [], []
                            for mi in range(MT):
                                msl = slice(mi * 128, (mi + 1) * 128)
                                ps = psp.tile([128, 512], F32, tag="mm", name="mm")
                                nc.tensor.matmul(ps[:], kct[0:64, msl], qhl[0:64, :],
                                                 start=True, stop=False)
                                nc.tensor.matmul(ps[:], kct[:, msl], qlh[:],
                                                 start=False, stop=True)
                                ef = exp_pool.tile([128, 512], F32, tag="ef",
                                                   name="ef", bufs=2)
                                eh = exp_pool.tile([128, 512], F32R, tag=f"eh{mi}",
                                                   name=f"eh{mi}", bufs=2)
                                el = exp_pool.tile([128, 512], F32R, tag=f"el{mi}",
                                                   name=f"el{mi}", bufs=2)
                                nc.scalar.activation(eh[:], ps[:], EXP)
                                nc.scalar.activation(ef[:], ps[:], EXP)
                                nc.vector.tensor_sub(el[:], ef[:], eh[:])
                                es_h.append(eh)
                                es_l.append(el)
                            po = pop.tile([65, 512], F32, tag="po", name="po")
                            idx = 0
                            for (vs, es) in ((va_h, es_h), (va_l, es_h),
                                             (va_h, es_l)):
                                for mi in range(MT):
                                    nc.tensor.matmul(
                                        po[:], vs[mi][:, h * 65:h * 65 + 65],
                                        es[mi][:], start=(idx == 0),
                                        stop=(idx == 23))
                                    idx += 1
                            rf = smp.tile([1, 512], F32, tag="rf", name="rf")
                            nc.vector.reciprocal(rf[:], po[64:65, :])
                            pbs = smp.tile([64, 512], F32, tag="pbs", name="pbs")
                            nc.gpsimd.partition_broadcast(pbs[:], rf[:])
                            af = smp.tile([64, 512], F32, tag="af", name="af")
                            nc.vector.tensor_mul(af[:], po[0:64, :], pbs[:])
                            afh = smp.tile([64, 512], F32R, tag="afh", name="afh")
                            afl = smp.tile([64, 512], F32R, tag="afl", name="afl")
                            nc.vector.tensor_copy(afh[:], af[:])
                            nc.vector.tensor_sub(afl[:], af[:], afh[:])
                            hsl2 = slice(h * 64, (h + 1) * 64)
                            nc.sync.dma_start(aob[hsl2, nsl], afh[:])
                            nc.sync.dma_start(aob[IS + h * 64:IS + (h + 1) * 64, nsl],
                                              afl[:])

            # -------- out = aoT.T @ Wo (3-pass split, partial) --------
            with nc.named_scope("oproj"), \
                 tc.tile_pool(name="wop", bufs=1) as wop, \
                 tc.tile_pool(name="aos", bufs=2) as aos:
                woh_sb = [wop.tile([128, QD], F32R, tag=f"woh{k}", name=f"woh{k}")
                          for k in range(IT)]
                wol_sb = [wop.tile([128, QD], F32R, tag=f"wol{k}", name=f"wol{k}")
                          for k in range(IT)]
                for k in range(IT):
                    ksl = slice(k * 128, (k + 1) * 128)
                    nc.sync.dma_start(woh_sb[k][:], woh[ksl, :])
                    nc.sync.dma_start(wol_sb[k][:], wol[ksl, :])
                for nt in range(N // 128):
                    tsl = slice(nt * 128, (nt + 1) * 128)
                    ah = [aos.tile([128, 128], F32R, tag=f"ah{k}", name=f"ah{k}")
                          for k in range(IT)]
                    al = [aos.tile([128, 128], F32R, tag=f"al{k}", name=f"al{k}")
                          for k in range(IT)]
                    for k in range(IT):
                        nc.sync.dma_start(ah[k][:],
                                          aob[k * 128:(k + 1) * 128, tsl])
                        nc.sync.dma_start(al[k][:],
                                          aob[IS + k * 128:IS + (k + 1) * 128,
                                              tsl])
                    ob = osp.tile([128, QD], F32, tag="ob", name="ob")
                    for half in range(QD // 512):
                        qsl = slice(half * 512, (half + 1) * 512)
                        ps = psp.tile([128, 512], F32, tag="mm", name="mm")
                        split_mm(ps,
                                 [t[:] for t in ah],
                                 [t[:] for t in al],
                                 [t[:, qsl] for t in woh_sb],
                                 [t[:, qsl] for t in wol_sb], IT)
                        nc.vector.tensor_copy(ob[:, qsl], ps[:])
                    nc.sync.dma_start(out[tsl, :], ob[:])
    nc.compile()
    return nc


_NC_CACHE = None


def kernel(x, context, Wq, Wk, Wv, Wo, bo, _trace=False):
    global _NC_CACHE, LAST_RESULTS
    x = np.asarray(x, np.float32)
    context = np.asarray(context, np.float32)
    scale = np.float32(DH ** -0.5)

    if _NC_CACHE is None:
        _NC_CACHE = build_nc()
    nc = _NC_CACHE

    def hl(a):
        a = np.ascontiguousarray(a, np.float32)
        hi = _r(a)
        return hi, (a - hi).astype(np.float32)

    in_maps = []
    for c in range(NC):
        b, g = c // 2, c % 2
        sl = slice(g * IS, (g + 1) * IS)
        m = {}
        m["xTh"], m["xTl"] = hl(x[b].T)
        m["cTh"], m["cTl"] = hl(context[b].T)
        m["wqh"], m["wql"] = hl(np.asarray(Wq, np.float32)[:, sl] * scale)
        m["wkh"], m["wkl"] = hl(np.asarray(Wk, np.float32)[:, sl])
        m["wvh"], m["wvl"] = hl(np.asarray(Wv, np.float32)[:, sl])
        m["woh"], m["wol"] = hl(np.asarray(Wo, np.float32)[sl, :])
        in_maps.append(m)
    res = run_bass_kernel_spmd(nc, in_maps, core_ids=list(range(NC)),
                               trace=_trace)
    LAST_RESULTS = res
    out = np.empty((B, N, QD), np.float32)
    bo32 = np.asarray(bo, np.float32)
    for b in range(B):
        out[b] = res.results[2 * b]["out"] + res.results[2 * b + 1]["out"] + bo32
    return out



# revision 4
# speedup vs baseline: 2.8057x; 2.8057x over previous
"""CrossAttention Trainium2 kernel, 8-core SPMD, bf16 single-pass matmuls.

Sharding: core c -> (batch b = c//2, head-group g = c%2).  Each core computes
8 of the 16 heads for one batch: q/k/v projections restricted to its
inner-dim slice [g*512:(g+1)*512], full attention for those heads, and a
partial output projection (contraction over its 512 inner dims).  Host casts
inputs to bf16 and sums the two partial fp32 outputs per batch + bias.

Tolerance is 2e-2 rel; bf16 matmuls (fp32 PSUM accumulation) land ~1e-3, so
every matmul is a single bf16 pass (1 PE cycle/row) instead of the 3-pass
fp32r hi/lo split - 3x less PE work, half the ACT exp work, and no hi/lo
vector traffic.

Per-core dataflow (everything SBUF-resident between the input loads and the
final out store; heads are packed in pairs onto 128-partition tiles):
  kT[512,1024]  = Wk_s^T-contracted from cT; tile t holds heads 2t,2t+1
  V  [1024,512] -> va tiles [128, 8*65]; col h*65+64 is 1.0 so the softmax
                  denominator rides the attn@v matmul
  per n-chunk jn (512 cols):
    qT[512,512]  (scale folded into Wq)
    per head h: simT[m,n] = kct_h^T q_h (K=64), exp on ACT -> bf16 es
                po[65,512] = va_h^T es (K=128, 8 passes)
                rf = recip(po[64]) -> gpsimd partition_broadcast -> pbs
                aoT rows of h = po[0:64] * pbs  (bf16)
    oproj(jn): out[n,1024] = aoT^T @ Wo per 128-row chunk (partial; host
               adds the pair of head-group results + bias)
"""
import sys

sys.path.insert(0, "/opt/trn_rl_repo")

import numpy as np
import ml_dtypes

import concourse.bacc as bacc
import concourse.mybir as mybir
import concourse.tile as tile
from concourse.bass_utils import run_bass_kernel_spmd

# bass_utils imports antenv.axon_hooks when trace=True; register a no-op stub
# if the antenv package in this image lacks it.
try:
    import antenv.axon_hooks  # noqa: F401
except ImportError:
    import types as _types

    _stub = _types.ModuleType("antenv.axon_hooks")
    _stub.get_axon_ntff_profile_hook = lambda: None
    _stub.set_axon_ntff_profile_hook = lambda h: None
    sys.modules["antenv.axon_hooks"] = _stub

F32 = mybir.dt.float32
BF16 = mybir.dt.bfloat16
EXP = mybir.ActivationFunctionType.Exp

B, N, M = 4, 2048, 1024
QD, CD = 1024, 768
HEADS, DH = 16, 64
INNER = HEADS * DH
HG = 8            # heads per core
IS = HG * DH      # inner slice per core = 512
NC = 8

KQ = QD // 128    # 8
KC = CD // 128    # 6
NJ = N // 512     # 4
MT = M // 128     # 8
IT = IS // 128    # 4

LAST_RESULTS = None  # stashed BassKernelResults for test.py introspection


def build_nc():
    nc = bacc.Bacc("TRN2", target_bir_lowering=False, debug=False, num_devices=NC)

    def din(name, shape):
        return nc.dram_tensor(name, shape, BF16, kind="ExternalInput").ap()

    xT = din("xT", [QD, N])
    cT = din("cT", [CD, M])
    wq = din("wq", [QD, IS])
    wk = din("wk", [CD, IS])
    wv = din("wv", [CD, IS])
    wo = din("wo", [IS, QD])
    out = nc.dram_tensor("out", [N, QD], F32, kind="ExternalOutput").ap()

    with tile.TileContext(nc) as tc:
        with tc.tile_pool(name="wp", bufs=1) as wp, \
             tc.tile_pool(name="xp", bufs=1) as xp, \
             tc.tile_pool(name="kv", bufs=1) as kvp, \
             tc.tile_pool(name="qp", bufs=1) as qp, \
             tc.tile_pool(name="ao", bufs=1) as aop, \
             tc.tile_pool(name="es", bufs=2) as esp, \
             tc.tile_pool(name="sm", bufs=2) as smp, \
             tc.tile_pool(name="os", bufs=2) as osp, \
             tc.tile_pool(name="ps", bufs=2, space="PSUM") as psp, \
             tc.tile_pool(name="pq", bufs=2, space="PSUM") as pqp, \
             tc.tile_pool(name="po", bufs=2, space="PSUM") as pop:

            # ---------------- resident input loads ----------------
            wq_sb = [wp.tile([128, IS], BF16, tag=f"wq{k}", name=f"wq{k}")
                     for k in range(KQ)]
            wk_sb = [wp.tile([128, IS], BF16, tag=f"wk{k}", name=f"wk{k}")
                     for k in range(KC)]
            wv_sb = [wp.tile([128, IS], BF16, tag=f"wv{k}", name=f"wv{k}")
                     for k in range(KC)]
            wo_sb = [wp.tile([128, QD], BF16, tag=f"wo{k}", name=f"wo{k}")
                     for k in range(IT)]
            xT_sb = [xp.tile([128, N], BF16, tag=f"xT{k}", name=f"xT{k}")
                     for k in range(KQ)]
            cT_sb = [xp.tile([128, M], BF16, tag=f"cT{k}", name=f"cT{k}")
                     for k in range(KC)]
            for k in range(KC):
                ksl = slice(k * 128, (k + 1) * 128)
                nc.sync.dma_start(cT_sb[k][:], cT[ksl, :])
                nc.sync.dma_start(wk_sb[k][:], wk[ksl, :])
                nc.sync.dma_start(wv_sb[k][:], wv[ksl, :])
            for k in range(KQ):
                ksl = slice(k * 128, (k + 1) * 128)
                nc.sync.dma_start(xT_sb[k][:], xT[ksl, :])
                nc.sync.dma_start(wq_sb[k][:], wq[ksl, :])
            for k in range(IT):
                nc.sync.dma_start(wo_sb[k][:], wo[k * 128:(k + 1) * 128, :])

            kct_sb = [kvp.tile([128, M], BF16, tag=f"kct{t}", name=f"kct{t}")
                      for t in range(IT)]
            va_sb = [kvp.tile([128, HG * 65], BF16, tag=f"va{mi}",
                              name=f"va{mi}") for mi in range(MT)]
            qT_sb = [qp.tile([128, N], BF16, tag=f"qT{t}", name=f"qT{t}")
                     for t in range(IT)]
            aoT_sb = [aop.tile([128, N], BF16, tag=f"aoT{t}", name=f"aoT{t}")
                      for t in range(IT)]

            # ---------------- K^T -> kct ----------------
            with nc.named_scope("kproj"):
                for jm in range(M // 512):
                    msl = slice(jm * 512, (jm + 1) * 512)
                    for mi in range(IT):
                        isl = slice(mi * 128, (mi + 1) * 128)
                        ps = pqp.tile([128, 512], F32, tag="pq", name="pq")
                        for k in range(KC):
                            nc.tensor.matmul(ps[:], wk_sb[k][:, isl],
                                             cT_sb[k][:, msl],
                                             start=(k == 0), stop=(k == KC - 1))
                        nc.vector.tensor_copy(kct_sb[mi][:, msl], ps[:])

            # ---------------- V -> va (ones col rides along) ----------------
            with nc.named_scope("vproj"):
                for mi in range(MT):
                    nc.vector.memset(va_sb[mi][:], 1.0)
                for mi in range(MT):
                    csl = slice(mi * 128, (mi + 1) * 128)
                    ps = pqp.tile([128, 512], F32, tag="pq", name="pq")
                    for k in range(KC):
                        nc.tensor.matmul(ps[:], cT_sb[k][:, csl], wv_sb[k][:],
                                         start=(k == 0), stop=(k == KC - 1))
                    vcol = va_sb[mi][:].rearrange("p (h c) -> p h c", c=65)
                    psv = ps[:].rearrange("p (h c) -> p h c", c=64)
                    nc.vector.tensor_copy(vcol[:, :, 0:64], psv[:])

            # ------------- per n-chunk: qproj -> attn -> oproj -------------
            for jn in range(NJ):
                nsl = slice(jn * 512, (jn + 1) * 512)
                with nc.named_scope("qproj"):
                    for mi in range(IT):
                        isl = slice(mi * 128, (mi + 1) * 128)
                        ps = pqp.tile([128, 512], F32, tag="pq", name="pq")
                        for k in range(KQ):
                            nc.tensor.matmul(ps[:], wq_sb[k][:, isl],
                                             xT_sb[k][:, nsl],
                                             start=(k == 0), stop=(k == KQ - 1))
                        nc.vector.tensor_copy(qT_sb[mi][:, nsl], ps[:])

                with nc.named_scope("attn"):
                    for h in range(HG):
                        hp, ro = h // 2, (h % 2) * 64
                        rsl = slice(ro, ro + 64)
                        # sim^T tiles: 2 psum banks per tile, exp amortized
                        es_t = []
                        for half in range(MT // 2):
                            ps = psp.tile([128, 1024], F32, tag="ps2",
                                          name="ps2")
                            for sub in range(2):
                                mi = 2 * half + sub
                                msl = slice(mi * 128, (mi + 1) * 128)
                                nc.tensor.matmul(
                                    ps[:, sub * 512:(sub + 1) * 512],
                                    kct_sb[hp][rsl, msl], qT_sb[hp][rsl, nsl],
                                    start=True, stop=True)
                            es = esp.tile([128, 1024], BF16, tag=f"es{half}",
                                          name=f"es{half}")
                            nc.scalar.activation(es[:], ps[:], EXP)
                            es_t.append(es)
                        po = pop.tile([65, 512], F32, tag="po", name="po")
                        for mi in range(MT):
                            nc.tensor.matmul(
                                po[:], va_sb[mi][:, h * 65:h * 65 + 65],
                                es_t[mi // 2][:, (mi % 2) * 512:
                                              (mi % 2) * 512 + 512],
                                start=(mi == 0), stop=(mi == MT - 1))
                        # reciprocal_approx_fast misreads PSUM inputs; stage
                        # the denominator row through SBUF first.
                        dn = smp.tile([1, 512], F32, tag="dn", name="dn")
                        nc.vector.tensor_copy(dn[:], po[64:65, :])
                        rf = smp.tile([1, 512], F32, tag="rf", name="rf")
                        nc.vector.reciprocal_approx_fast(out=rf[:], in_=dn[:])
                        pbs = smp.tile([64, 512], F32, tag="pbs", name="pbs")
                        nc.gpsimd.partition_broadcast(pbs[:], rf[:])
                        nc.vector.tensor_mul(aoT_sb[hp][rsl, nsl],
                                             po[0:64, :], pbs[:])

                with nc.named_scope("oproj"):
                    for nt in range(4):
                        tsl = slice(jn * 512 + nt * 128, jn * 512 + nt * 128 + 128)
                        ob = osp.tile([128, QD], F32, tag="ob", name="ob")
                        for half in range(QD // 512):
                            qsl = slice(half * 512, (half + 1) * 512)
                            ps = pqp.tile([128, 512], F32, tag="pq", name="pq")
                            for k in range(IT):
                                nc.tensor.matmul(ps[:], aoT_sb[k][:, tsl],
                                                 wo_sb[k][:, qsl],
                                                 start=(k == 0),
                                                 stop=(k == IT - 1))
                            nc.vector.tensor_copy(ob[:, qsl], ps[:])
                        nc.sync.dma_start(out[tsl, :], ob[:])
    nc.compile()
    return nc


_NC_CACHE = None


def kernel(x, context, Wq, Wk, Wv, Wo, bo, _trace=False):
    global _NC_CACHE, LAST_RESULTS
    x = np.asarray(x, np.float32)
    context = np.asarray(context, np.float32)
    scale = np.float32(DH ** -0.5)

    if _NC_CACHE is None:
        _NC_CACHE = build_nc()
    nc = _NC_CACHE

    bf = lambda a: np.ascontiguousarray(a).astype(ml_dtypes.bfloat16)

    in_maps = []
    for c in range(NC):
        b, g = c // 2, c % 2
        sl = slice(g * IS, (g + 1) * IS)
        m = {
            "xT": bf(x[b].T),
            "cT": bf(context[b].T),
            "wq": bf(np.asarray(Wq, np.float32)[:, sl] * scale),
            "wk": bf(np.asarray(Wk, np.float32)[:, sl]),
            "wv": bf(np.asarray(Wv, np.float32)[:, sl]),
            "wo": bf(np.asarray(Wo, np.float32)[sl, :]),
        }
        in_maps.append(m)
    res = run_bass_kernel_spmd(nc, in_maps, core_ids=list(range(NC)),
                               trace=_trace)
    LAST_RESULTS = res
    out = np.empty((B, N, QD), np.float32)
    bo32 = np.asarray(bo, np.float32)
    for b in range(B):
        out[b] = res.results[2 * b]["out"] + res.results[2 * b + 1]["out"] + bo32
    return out


# revision 10
# speedup vs baseline: 3.1925x; 1.1378x over previous
"""CrossAttention Trainium2 kernel, 8-core SPMD, bf16 single-pass matmuls.

Sharding: core c -> (batch b = c//2, head-group g = c%2).  Each core computes
8 of the 16 heads for one batch: q/k/v projections restricted to its
inner-dim slice [g*512:(g+1)*512], full attention for those heads, and a
partial output projection (contraction over its 512 inner dims).  Host casts
inputs to bf16 and sums the two partial fp32 outputs per batch + bias.

Tolerance is 2e-2 rel; bf16 matmuls (fp32 PSUM accumulation) land ~3e-3, so
every matmul is a single bf16 pass (1 PE cycle/row) instead of the 3-pass
fp32r hi/lo split - 3x less PE work, half the ACT exp work, and no hi/lo
vector traffic.  (fp8 DoubleRow was tried and measured no faster than bf16
on this compile path, with error past the gate - see kernel_fp8_attempt.)

Per-core dataflow (everything SBUF-resident between the input loads and the
final out store; heads are packed in pairs onto 128-partition tiles):
  kT[512,1024]  = Wk-contracted from cT; tile t holds heads 2t,2t+1
  V  [1024,512] -> va tiles [128, 8*65]; col h*65+64 is 1.0 so the softmax
                  denominator rides the attn@v matmul
  per n-chunk jn (512 cols):
    qT[512,512]  (scale folded into Wq)
    per head h: simT[m,n] = kct_h^T q_h (K=64), exp on ACT -> bf16 es
                po[65,512] = va_h^T es (K=128, 8 passes)
                rf = recip(po[64]) -> gpsimd partition_broadcast -> pbs
                aoT rows of h = po[0:64] * pbs  (bf16)
    oproj(jn): out[n,1024] = aoT^T @ Wo per 128-row chunk (partial; host
               adds the pair of head-group results + bias)
"""
import sys

sys.path.insert(0, "/opt/trn_rl_repo")

import numpy as np
import ml_dtypes

import concourse.bacc as bacc
import concourse.mybir as mybir
import concourse.tile as tile
from concourse.bass_utils import run_bass_kernel_spmd

# bass_utils imports antenv.axon_hooks when trace=True; register a no-op stub
# if the antenv package in this image lacks it.
try:
    import antenv.axon_hooks  # noqa: F401
except ImportError:
    import types as _types

    _stub = _types.ModuleType("antenv.axon_hooks")
    _stub.get_axon_ntff_profile_hook = lambda: None
    _stub.set_axon_ntff_profile_hook = lambda h: None
    sys.modules["antenv.axon_hooks"] = _stub

F32 = mybir.dt.float32
BF16 = mybir.dt.bfloat16
EXP = mybir.ActivationFunctionType.Exp

B, N, M = 4, 2048, 1024
QD, CD = 1024, 768
HEADS, DH = 16, 64
INNER = HEADS * DH
HG = 8            # heads per core
IS = HG * DH      # inner slice per core = 512
NC = 8

KQ = QD // 128    # 8
KC = CD // 128    # 6
NJ = N // 512     # 4
MT = M // 128     # 8
IT = IS // 128    # 4

LAST_RESULTS = None  # stashed BassKernelResults for test.py introspection


def build_nc():
    nc = bacc.Bacc("TRN2", target_bir_lowering=False, debug=False, num_devices=NC)

    def din(name, shape):
        return nc.dram_tensor(name, shape, BF16, kind="ExternalInput").ap()

    xT = din("xT", [QD, N])
    cT = din("cT", [CD, M])
    wq = din("wq", [QD, IS])
    wk = din("wk", [CD, IS])
    wv = din("wv", [CD, IS])
    wo = din("wo", [IS, QD])
    out = nc.dram_tensor("out", [N, QD], F32, kind="ExternalOutput").ap()

    with tile.TileContext(nc) as tc:
        with tc.tile_pool(name="wp", bufs=1) as wp, \
             tc.tile_pool(name="xp", bufs=1) as xp, \
             tc.tile_pool(name="kv", bufs=1) as kvp, \
             tc.tile_pool(name="qp", bufs=2) as qp, \
             tc.tile_pool(name="ao", bufs=2) as aop, \
             tc.tile_pool(name="es", bufs=2) as esp, \
             tc.tile_pool(name="sm", bufs=2) as smp, \
             tc.tile_pool(name="os", bufs=2) as osp, \
             tc.tile_pool(name="ps", bufs=2, space="PSUM") as psp, \
             tc.tile_pool(name="pq", bufs=2, space="PSUM") as pqp, \
             tc.tile_pool(name="po", bufs=2, space="PSUM") as pop:

            # ---------------- resident input loads ----------------
            wq_sb = [wp.tile([128, IS], BF16, tag=f"wq{k}", name=f"wq{k}")
                     for k in range(KQ)]
            wk_sb = [wp.tile([128, IS], BF16, tag=f"wk{k}", name=f"wk{k}")
                     for k in range(KC)]
            wv_sb = [wp.tile([128, IS], BF16, tag=f"wv{k}", name=f"wv{k}")
                     for k in range(KC)]
            wo_sb = [wp.tile([128, QD], BF16, tag=f"wo{k}", name=f"wo{k}")
                     for k in range(IT)]
            xT_sb = [xp.tile([128, N], BF16, tag=f"xT{k}", name=f"xT{k}")
                     for k in range(KQ)]
            cT_sb = [xp.tile([128, M], BF16, tag=f"cT{k}", name=f"cT{k}")
                     for k in range(KC)]
            for k in range(KC):
                ksl = slice(k * 128, (k + 1) * 128)
                nc.sync.dma_start(wk_sb[k][:], wk[ksl, :])
                nc.sync.dma_start(cT_sb[k][:], cT[ksl, :])
            for k in range(KC):
                ksl = slice(k * 128, (k + 1) * 128)
                nc.sync.dma_start(wv_sb[k][:], wv[ksl, :])
            for k in range(KQ):
                ksl = slice(k * 128, (k + 1) * 128)
                nc.sync.dma_start(xT_sb[k][:], xT[ksl, :])
                nc.sync.dma_start(wq_sb[k][:], wq[ksl, :])
            for k in range(IT):
                nc.sync.dma_start(wo_sb[k][:], wo[k * 128:(k + 1) * 128, :])

            kct_sb = [kvp.tile([128, M], BF16, tag=f"kct{t}", name=f"kct{t}")
                      for t in range(IT)]
            va_sb = [kvp.tile([128, HG * 65], BF16, tag=f"va{mi}",
                              name=f"va{mi}") for mi in range(MT)]


            # ---------------- K^T -> kct ----------------
            with nc.named_scope("kproj"):
                for jm in range(M // 512):
                    msl = slice(jm * 512, (jm + 1) * 512)
                    for mi in range(IT):
                        isl = slice(mi * 128, (mi + 1) * 128)
                        ps = pqp.tile([128, 512], F32, tag="pq", name="pq")
                        for k in range(KC):
                            nc.tensor.matmul(ps[:], wk_sb[k][:, isl],
                                             cT_sb[k][:, msl],
                                             start=(k == 0), stop=(k == KC - 1))
                        nc.vector.tensor_copy(kct_sb[mi][:, msl], ps[:])

            # ---------------- V -> va (ones col rides along) ----------------
            with nc.named_scope("vproj"):
                for mi in range(MT):
                    nc.vector.memset(va_sb[mi][:], 1.0)
                for mi in range(MT):
                    csl = slice(mi * 128, (mi + 1) * 128)
                    ps = pqp.tile([128, 512], F32, tag="pq", name="pq")
                    for k in range(KC):
                        nc.tensor.matmul(ps[:], cT_sb[k][:, csl], wv_sb[k][:],
                                         start=(k == 0), stop=(k == KC - 1))
                    vcol = va_sb[mi][:].rearrange("p (h c) -> p h c", c=65)
                    psv = ps[:].rearrange("p (h c) -> p h c", c=64)
                    nc.vector.tensor_copy(vcol[:, :, 0:64], psv[:])

            # ---- per n-chunk, software-pipelined ----
            # sims run one head ahead of attn@v; qproj(jn+1) and oproj(jn-1)
            # chunks are spliced between heads as always-ready PE filler so
            # the PE never stalls on the ACT exp backlog.
            qts = {}   # jn -> [qT tiles]
            aos = {}   # jn -> [aoT tiles]

            def qproj_chunk(jn, mi):
                nsl = slice(jn * 512, (jn + 1) * 512)
                isl = slice(mi * 128, (mi + 1) * 128)
                with nc.named_scope("qproj"):
                    ps = pqp.tile([128, 512], F32, tag="pq", name="pq")
                    for k in range(KQ):
                        nc.tensor.matmul(ps[:], wq_sb[k][:, isl],
                                         xT_sb[k][:, nsl],
                                         start=(k == 0), stop=(k == KQ - 1))
                    nc.vector.tensor_copy(qts[jn][mi][:], ps[:])

            def oproj_chunk(jn, nt):
                tsl = slice(nt * 128, (nt + 1) * 128)
                osl = slice(jn * 512 + nt * 128, jn * 512 + nt * 128 + 128)
                with nc.named_scope("oproj"):
                    ob = osp.tile([128, QD], F32, tag="ob", name="ob")
                    for half in range(QD // 512):
                        qsl = slice(half * 512, (half + 1) * 512)
                        ps = pqp.tile([128, 512], F32, tag="pq", name="pq")
                        for k in range(IT):
                            nc.tensor.matmul(ps[:], aos[jn][k][:, tsl],
                                             wo_sb[k][:, qsl],
                                             start=(k == 0), stop=(k == IT - 1))
                        nc.vector.tensor_copy(ob[:, qsl], ps[:])
                    nc.sync.dma_start(out[osl, :], ob[:])

            def sim_exp(jn, h):
                hp, ro = h // 2, (h % 2) * 64
                rsl = slice(ro, ro + 64)
                es_t = []
                for half in range(MT // 2):
                    ps = psp.tile([128, 1024], F32, tag="ps2", name="ps2")
                    for sub in range(2):
                        mi = 2 * half + sub
                        msl = slice(mi * 128, (mi + 1) * 128)
                        nc.tensor.matmul(ps[:, sub * 512:(sub + 1) * 512],
                                         kct_sb[hp][rsl, msl],
                                         qts[jn][hp][rsl, :],
                                         start=True, stop=True)
                    es = esp.tile([128, 1024], BF16, tag=f"es{half}",
                                  name=f"es{half}", bufs=3)
                    nc.scalar.activation(es[:], ps[:], EXP)
                    es_t.append(es)
                return es_t

            def attn_tail(jn, h, es_t):
                hp, ro = h // 2, (h % 2) * 64
                rsl = slice(ro, ro + 64)
                po = pop.tile([65, 512], F32, tag="po", name="po")
                for mi in range(MT):
                    nc.tensor.matmul(po[:],
                                     va_sb[mi][:, h * 65:h * 65 + 65],
                                     es_t[mi // 2][:, (mi % 2) * 512:
                                                   (mi % 2) * 512 + 512],
                                     start=(mi == 0), stop=(mi == MT - 1))
                # reciprocal_approx_fast misreads PSUM inputs; stage the
                # denominator row through SBUF first.
                dn = smp.tile([1, 512], F32, tag="dn", name="dn")
                nc.vector.tensor_copy(dn[:], po[64:65, :])
                rf = smp.tile([1, 512], F32, tag="rf", name="rf")
                nc.vector.reciprocal_approx_fast(out=rf[:], in_=dn[:])
                pbs = smp.tile([64, 512], F32, tag="pbs", name="pbs")
                nc.gpsimd.partition_broadcast(pbs[:], rf[:])
                nc.vector.tensor_mul(aos[jn][hp][rsl, :], po[0:64, :], pbs[:])

            qts[0] = [qp.tile([128, 512], BF16, tag=f"qT{t}", name=f"qT{t}")
                      for t in range(IT)]
            for mi in range(IT):
                qproj_chunk(0, mi)
            for jn in range(NJ):
                aos[jn] = [aop.tile([128, 512], BF16, tag=f"aoT{t}",
                                    name=f"aoT{t}") for t in range(IT)]
                fillers = []
                if jn + 1 < NJ:
                    qts[jn + 1] = [qp.tile([128, 512], BF16, tag=f"qT{t}",
                                           name=f"qT{t}") for t in range(IT)]
                    fillers += [(qproj_chunk, jn + 1, mi) for mi in range(IT)]
                if jn > 0:
                    fillers += [(oproj_chunk, jn - 1, nt) for nt in range(4)]
                with nc.named_scope("attn"):
                    es_cur = sim_exp(jn, 0)
                    for h in range(HG):
                        es_nxt = sim_exp(jn, h + 1) if h + 1 < HG else None
                        attn_tail(jn, h, es_cur)
                        es_cur = es_nxt
                        if fillers:
                            fn, a1, a2 = fillers.pop(0)
                            fn(a1, a2)
            for nt in range(4):
                oproj_chunk(NJ - 1, nt)
    nc.compile()
    return nc


_NC_CACHE = None


def kernel(x, context, Wq, Wk, Wv, Wo, bo, _trace=False):
    global _NC_CACHE, LAST_RESULTS
    x = np.asarray(x, np.float32)
    context = np.asarray(context, np.float32)
    scale = np.float32(DH ** -0.5)

    if _NC_CACHE is None:
        _NC_CACHE = build_nc()
    nc = _NC_CACHE

    bf = lambda a: np.ascontiguousarray(a).astype(ml_dtypes.bfloat16)

    in_maps = []
    for c in range(NC):
        b, g = c // 2, c % 2
        sl = slice(g * IS, (g + 1) * IS)
        m = {
            "xT": bf(x[b].T),
            "cT": bf(context[b].T),
            "wq": bf(np.asarray(Wq, np.float32)[:, sl] * scale),
            "wk": bf(np.asarray(Wk, np.float32)[:, sl]),
            "wv": bf(np.asarray(Wv, np.float32)[:, sl]),
            "wo": bf(np.asarray(Wo, np.float32)[sl, :]),
        }
        in_maps.append(m)
    res = run_bass_kernel_spmd(nc, in_maps, core_ids=list(range(NC)),
                               trace=_trace)
    LAST_RESULTS = res
    out = np.empty((B, N, QD), np.float32)
    bo32 = np.asarray(bo, np.float32)
    for b in range(B):
        out[b] = res.results[2 * b]["out"] + res.results[2 * b + 1]["out"] + bo32
    return out


# revision 11
# speedup vs baseline: 3.2524x; 1.0188x over previous
"""CrossAttention Trainium2 kernel, 8-core SPMD, bf16 single-pass matmuls.

Sharding: core c -> (batch b = c//2, head-group g = c%2).  Each core computes
8 of the 16 heads for one batch: q/k/v projections restricted to its
inner-dim slice [g*512:(g+1)*512], full attention for those heads, and a
partial output projection (contraction over its 512 inner dims).  Host casts
inputs to bf16 and sums the two partial fp32 outputs per batch + bias.

Tolerance is 2e-2 rel; bf16 matmuls (fp32 PSUM accumulation) land ~3e-3, so
every matmul is a single bf16 pass (1 PE cycle/row) instead of the 3-pass
fp32r hi/lo split - 3x less PE work, half the ACT exp work, and no hi/lo
vector traffic.  (fp8 DoubleRow was tried and measured no faster than bf16
on this compile path, with error past the gate - see kernel_fp8_attempt.)

Per-core dataflow (everything SBUF-resident between the input loads and the
final out store; heads are packed in pairs onto 128-partition tiles):
  kT[512,1024]  = Wk-contracted from cT; tile t holds heads 2t,2t+1
  V  [1024,512] -> va tiles [128, 8*65]; col h*65+64 is 1.0 so the softmax
                  denominator rides the attn@v matmul
  per n-chunk jn (512 cols):
    qT[512,512]  (scale folded into Wq)
    per head h: simT[m,n] = kct_h^T q_h (K=64), exp on ACT -> bf16 es
                po[65,512] = va_h^T es (K=128, 8 passes)
                rf = recip(po[64]) -> gpsimd partition_broadcast -> pbs
                aoT rows of h = po[0:64] * pbs  (bf16)
    oproj(jn): out[n,1024] = aoT^T @ Wo per 128-row chunk (partial; host
               adds the pair of head-group results + bias)
"""
import sys

sys.path.insert(0, "/opt/trn_rl_repo")

import numpy as np
import ml_dtypes

import concourse.bacc as bacc
import concourse.mybir as mybir
import concourse.tile as tile
from concourse.bass_utils import run_bass_kernel_spmd

# bass_utils imports antenv.axon_hooks when trace=True; register a no-op stub
# if the antenv package in this image lacks it.
try:
    import antenv.axon_hooks  # noqa: F401
except ImportError:
    import types as _types

    _stub = _types.ModuleType("antenv.axon_hooks")
    _stub.get_axon_ntff_profile_hook = lambda: None
    _stub.set_axon_ntff_profile_hook = lambda h: None
    sys.modules["antenv.axon_hooks"] = _stub

F32 = mybir.dt.float32
BF16 = mybir.dt.bfloat16
EXP = mybir.ActivationFunctionType.Exp

B, N, M = 4, 2048, 1024
QD, CD = 1024, 768
HEADS, DH = 16, 64
INNER = HEADS * DH
HG = 8            # heads per core
IS = HG * DH      # inner slice per core = 512
NC = 8

KQ = QD // 128    # 8
KC = CD // 128    # 6
NJ = N // 512     # 4
MT = M // 128     # 8
IT = IS // 128    # 4

LAST_RESULTS = None  # stashed BassKernelResults for test.py introspection


def build_nc():
    nc = bacc.Bacc("TRN2", target_bir_lowering=False, debug=False, num_devices=NC)

    def din(name, shape):
        return nc.dram_tensor(name, shape, BF16, kind="ExternalInput").ap()

    xT = din("xT", [QD, N])
    cT = din("cT", [CD, M])
    wq = din("wq", [QD, IS])
    wk = din("wk", [CD, IS])
    wv = din("wv", [CD, IS])
    wo = din("wo", [IS, QD])
    out = nc.dram_tensor("out", [N, QD], F32, kind="ExternalOutput").ap()

    with tile.TileContext(nc) as tc:
        with tc.tile_pool(name="wp", bufs=1) as wp, \
             tc.tile_pool(name="xp", bufs=1) as xp, \
             tc.tile_pool(name="kv", bufs=1) as kvp, \
             tc.tile_pool(name="qp", bufs=2) as qp, \
             tc.tile_pool(name="ao", bufs=2) as aop, \
             tc.tile_pool(name="es", bufs=2) as esp, \
             tc.tile_pool(name="sm", bufs=2) as smp, \
             tc.tile_pool(name="os", bufs=2) as osp, \
             tc.tile_pool(name="ps", bufs=2, space="PSUM") as psp, \
             tc.tile_pool(name="pq", bufs=2, space="PSUM") as pqp, \
             tc.tile_pool(name="po", bufs=2, space="PSUM") as pop:

            # ---------------- resident input loads ----------------
            wq_sb = [wp.tile([128, IS], BF16, tag=f"wq{k}", name=f"wq{k}")
                     for k in range(KQ)]
            wk_sb = [wp.tile([128, IS], BF16, tag=f"wk{k}", name=f"wk{k}")
                     for k in range(KC)]
            wv_sb = [wp.tile([128, IS], BF16, tag=f"wv{k}", name=f"wv{k}")
                     for k in range(KC)]
            wo_sb = [wp.tile([128, QD], BF16, tag=f"wo{k}", name=f"wo{k}")
                     for k in range(IT)]
            xT_sb = [xp.tile([128, N], BF16, tag=f"xT{k}", name=f"xT{k}")
                     for k in range(KQ)]
            cT_sb = [xp.tile([128, M], BF16, tag=f"cT{k}", name=f"cT{k}")
                     for k in range(KC)]
            for k in range(KC):
                ksl = slice(k * 128, (k + 1) * 128)
                nc.sync.dma_start(wk_sb[k][:], wk[ksl, :])
                nc.sync.dma_start(cT_sb[k][:], cT[ksl, :])
            for k in range(KC):
                ksl = slice(k * 128, (k + 1) * 128)
                nc.sync.dma_start(wv_sb[k][:], wv[ksl, :])
            for k in range(KQ):
                ksl = slice(k * 128, (k + 1) * 128)
                nc.sync.dma_start(wq_sb[k][:], wq[ksl, :])
            for jn in range(NJ):
                nsl = slice(jn * 512, (jn + 1) * 512)
                for k in range(KQ):
                    ksl = slice(k * 128, (k + 1) * 128)
                    nc.sync.dma_start(xT_sb[k][:, nsl], xT[ksl, nsl])
            for k in range(IT):
                nc.sync.dma_start(wo_sb[k][:], wo[k * 128:(k + 1) * 128, :])

            kct_sb = [kvp.tile([128, M], BF16, tag=f"kct{t}", name=f"kct{t}")
                      for t in range(IT)]
            va_sb = [kvp.tile([128, HG * 65], BF16, tag=f"va{mi}",
                              name=f"va{mi}") for mi in range(MT)]


            # ---------------- K^T -> kct ----------------
            with nc.named_scope("kproj"):
                for jm in range(M // 512):
                    msl = slice(jm * 512, (jm + 1) * 512)
                    for mi in range(IT):
                        isl = slice(mi * 128, (mi + 1) * 128)
                        ps = pqp.tile([128, 512], F32, tag="pq", name="pq")
                        for k in range(KC):
                            nc.tensor.matmul(ps[:], wk_sb[k][:, isl],
                                             cT_sb[k][:, msl],
                                             start=(k == 0), stop=(k == KC - 1))
                        nc.vector.tensor_copy(kct_sb[mi][:, msl], ps[:])

            # ---------------- V -> va (ones col rides along) ----------------
            with nc.named_scope("vproj"):
                for mi in range(MT):
                    nc.vector.memset(va_sb[mi][:], 1.0)
                for mi in range(MT):
                    csl = slice(mi * 128, (mi + 1) * 128)
                    ps = pqp.tile([128, 512], F32, tag="pq", name="pq")
                    for k in range(KC):
                        nc.tensor.matmul(ps[:], cT_sb[k][:, csl], wv_sb[k][:],
                                         start=(k == 0), stop=(k == KC - 1))
                    vcol = va_sb[mi][:].rearrange("p (h c) -> p h c", c=65)
                    psv = ps[:].rearrange("p (h c) -> p h c", c=64)
                    nc.vector.tensor_copy(vcol[:, :, 0:64], psv[:])

            # ---- per n-chunk, software-pipelined ----
            # sims run one head ahead of attn@v; qproj(jn+1) and oproj(jn-1)
            # chunks are spliced between heads as always-ready PE filler so
            # the PE never stalls on the ACT exp backlog.
            qts = {}   # jn -> [qT tiles]
            aos = {}   # jn -> [aoT tiles]

            def qproj_chunk(jn, mi):
                nsl = slice(jn * 512, (jn + 1) * 512)
                isl = slice(mi * 128, (mi + 1) * 128)
                with nc.named_scope("qproj"):
                    ps = pqp.tile([128, 512], F32, tag="pq", name="pq")
                    for k in range(KQ):
                        nc.tensor.matmul(ps[:], wq_sb[k][:, isl],
                                         xT_sb[k][:, nsl],
                                         start=(k == 0), stop=(k == KQ - 1))
                    nc.vector.tensor_copy(qts[jn][mi][:], ps[:])

            def oproj_chunk(jn, nt):
                tsl = slice(nt * 128, (nt + 1) * 128)
                osl = slice(jn * 512 + nt * 128, jn * 512 + nt * 128 + 128)
                with nc.named_scope("oproj"):
                    ob = osp.tile([128, QD], F32, tag="ob", name="ob")
                    for half in range(QD // 512):
                        qsl = slice(half * 512, (half + 1) * 512)
                        ps = pqp.tile([128, 512], F32, tag="pq", name="pq")
                        for k in range(IT):
                            nc.tensor.matmul(ps[:], aos[jn][k][:, tsl],
                                             wo_sb[k][:, qsl],
                                             start=(k == 0), stop=(k == IT - 1))
                        nc.vector.tensor_copy(ob[:, qsl], ps[:])
                        nc.sync.dma_start(out[osl, qsl], ob[:, qsl])

            def sim_exp(jn, h):
                hp, ro = h // 2, (h % 2) * 64
                rsl = slice(ro, ro + 64)
                es_t = []
                for half in range(MT // 2):
                    ps = psp.tile([128, 1024], F32, tag="ps2", name="ps2")
                    for sub in range(2):
                        mi = 2 * half + sub
                        msl = slice(mi * 128, (mi + 1) * 128)
                        nc.tensor.matmul(ps[:, sub * 512:(sub + 1) * 512],
                                         kct_sb[hp][rsl, msl],
                                         qts[jn][hp][rsl, :],
                                         start=True, stop=True)
                    es = esp.tile([128, 1024], BF16, tag=f"es{half}",
                                  name=f"es{half}", bufs=3)
                    nc.scalar.activation(es[:], ps[:], EXP)
                    es_t.append(es)
                return es_t

            def attn_tail(jn, h, es_t):
                hp, ro = h // 2, (h % 2) * 64
                rsl = slice(ro, ro + 64)
                po = pop.tile([65, 512], F32, tag="po", name="po")
                for mi in range(MT):
                    nc.tensor.matmul(po[:],
                                     va_sb[mi][:, h * 65:h * 65 + 65],
                                     es_t[mi // 2][:, (mi % 2) * 512:
                                                   (mi % 2) * 512 + 512],
                                     start=(mi == 0), stop=(mi == MT - 1))
                # reciprocal_approx_fast misreads PSUM inputs; stage the
                # denominator row through SBUF first.
                dn = smp.tile([1, 512], F32, tag="dn", name="dn")
                nc.vector.tensor_copy(dn[:], po[64:65, :])
                rf = smp.tile([1, 512], F32, tag="rf", name="rf")
                nc.vector.reciprocal_approx_fast(out=rf[:], in_=dn[:])
                pbs = smp.tile([64, 512], F32, tag="pbs", name="pbs")
                nc.gpsimd.partition_broadcast(pbs[:], rf[:])
                nc.vector.tensor_mul(aos[jn][hp][rsl, :], po[0:64, :], pbs[:])

            qts[0] = [qp.tile([128, 512], BF16, tag=f"qT{t}", name=f"qT{t}")
                      for t in range(IT)]
            for mi in range(IT):
                qproj_chunk(0, mi)
            for jn in range(NJ):
                aos[jn] = [aop.tile([128, 512], BF16, tag=f"aoT{t}",
                                    name=f"aoT{t}") for t in range(IT)]
                fillers = []
                if jn + 1 < NJ:
                    qts[jn + 1] = [qp.tile([128, 512], BF16, tag=f"qT{t}",
                                           name=f"qT{t}") for t in range(IT)]
                    fillers += [(qproj_chunk, jn + 1, mi) for mi in range(IT)]
                if jn > 0:
                    fillers += [(oproj_chunk, jn - 1, nt) for nt in range(4)]
                nfill = len(fillers)
                with nc.named_scope("attn"):
                    es_cur = sim_exp(jn, 0)
                    for h in range(HG):
                        es_nxt = sim_exp(jn, h + 1) if h + 1 < HG else None
                        attn_tail(jn, h, es_cur)
                        es_cur = es_nxt
                        if fillers and h >= HG - nfill:
                            fn, a1, a2 = fillers.pop(0)
                            fn(a1, a2)
            for nt in range(4):
                oproj_chunk(NJ - 1, nt)
    nc.compile()
    return nc


_NC_CACHE = None


def kernel(x, context, Wq, Wk, Wv, Wo, bo, _trace=False):
    global _NC_CACHE, LAST_RESULTS
    x = np.asarray(x, np.float32)
    context = np.asarray(context, np.float32)
    scale = np.float32(DH ** -0.5)

    if _NC_CACHE is None:
        _NC_CACHE = build_nc()
    nc = _NC_CACHE

    bf = lambda a: np.ascontiguousarray(a).astype(ml_dtypes.bfloat16)

    in_maps = []
    for c in range(NC):
        b, g = c // 2, c % 2
        sl = slice(g * IS, (g + 1) * IS)
        m = {
            "xT": bf(x[b].T),
            "cT": bf(context[b].T),
            "wq": bf(np.asarray(Wq, np.float32)[:, sl] * scale),
            "wk": bf(np.asarray(Wk, np.float32)[:, sl]),
            "wv": bf(np.asarray(Wv, np.float32)[:, sl]),
            "wo": bf(np.asarray(Wo, np.float32)[sl, :]),
        }
        in_maps.append(m)
    res = run_bass_kernel_spmd(nc, in_maps, core_ids=list(range(NC)),
                               trace=_trace)
    LAST_RESULTS = res
    out = np.empty((B, N, QD), np.float32)
    bo32 = np.asarray(bo, np.float32)
    for b in range(B):
        out[b] = res.results[2 * b]["out"] + res.results[2 * b + 1]["out"] + bo32
    return out
